# revision 8
# baseline (speedup 1.0000x reference)
# Trainium2 Bass kernel v2 for nn_Lowrank_Spattention (sparse_attention).
#
# Key change vs v1: z and x are transposed AND cast to bf16 ON HOST,
# so attn = z@M and xv = x@Wv run directly with DMA'd f32r tiles as lhsT —
# no PE transposes of z/x, no PSUM->SBUF copies for them, no bf16 casts.
#
# Per core (batch element), n=8192 tokens, f=256, h=4 heads, r=64 ranks:
#   attn = z @ M            (M = Wq_h K_h^T/sqrt(d) folded, f32r)
#   E    = exp(attn)        (bf16 resident, row layout)
#   xv   = sa * (x @ Wv)    (f32r resident, + sa*bv if bias)
#   G    = Eh^T @ [xv | rs] (PSUM accum over all chunks; Eh = E/rowsum)
#   PS   = (sb/sa) * G_blockdiag / colsum   (tiny finalize)
#   out  = xv + E @ PS      (pass B; eT via PE transpose)
#
# aux cols 256..259 of xv_res hold per-head rowsums rs so that
# G[:, 256+h] = sum_n Eh[n,(h,r)]*rs[n,h] = colsum_E for rows of head h.

import math
import os

import numpy as np

import concourse.bass as bass
import concourse.mybir as mybir
import concourse.tile as tile
from concourse import bacc

B, N, DIM = 8, 8192, 256
HEAD, RANK, HDIM = 4, 64, 64
NCORES = 8
CHUNK = 128
NCHUNK = N // CHUNK          # 64
SUPER = 8                    # chunks per staging super-chunk
NSUPER = NCHUNK // SUPER     # 8
XW = DIM + HEAD              # 260: [xv | rs0..rs3]

F32 = mybir.dt.float32
F32R = mybir.dt.float32r
BF16 = mybir.dt.bfloat16
Exp = mybir.ActivationFunctionType.Exp


def build_body(tc, outs, ins):
    nc = tc.nc
    zt, xt = ins["zt"], ins["xt"]
    out = outs["out"]
    has_ab = ins.get("ab_row") is not None
    has_bias = bool(ins.get("has_bias", False))
    nbufs = 3

    with (
        tc.tile_pool(name="consts", bufs=1) as consts,
        tc.tile_pool(name="resident", bufs=1) as resident,
        tc.tile_pool(name="prefetch", bufs=1) as prefetch,
    ):
        # ---- prefetch the first chunks of zt/xt before anything else, so
        # the PE pipeline starts as early as possible ----
        zt_m = zt.rearrange("(t p) (s c) -> s p t c", p=128, c=SUPER * CHUNK)
        xt_m = xt.rearrange("(t p) (s c) -> s p t c", p=128, c=SUPER * CHUNK)
        zt_s0 = prefetch.tile([128, 2, SUPER * CHUNK], BF16)
        nc.sync.dma_start(out=zt_s0[:, :, 0:CHUNK], in_=zt_m[0, :, :, 0:CHUNK])
        mq_s = consts.tile([128, 2, DIM], BF16)
        nc.sync.dma_start(out=mq_s, in_=ins["mq"].rearrange("(t p) n -> p t n", p=128))
        xt_s0 = prefetch.tile([128, 2, SUPER * CHUNK], BF16)
        nc.sync.dma_start(out=xt_s0[:, :, 0:CHUNK], in_=xt_m[0, :, :, 0:CHUNK])
        swv_s = consts.tile([128, 2, DIM], BF16)
        nc.sync.dma_start(out=swv_s, in_=ins["swv"].rearrange("(t p) n -> p t n", p=128))
        c4 = 4 * CHUNK
        nc.sync.dma_start(out=zt_s0[:, :, CHUNK:c4], in_=zt_m[0, :, :, CHUNK:c4])
        nc.sync.dma_start(out=xt_s0[:, :, CHUNK:c4], in_=xt_m[0, :, :, CHUNK:c4])
        nc.sync.dma_start(out=zt_s0[:, :, c4:], in_=zt_m[0, :, :, c4:])
        nc.sync.dma_start(out=xt_s0[:, :, c4:], in_=xt_m[0, :, :, c4:])

        # ---- constants ----
        ident_bf = consts.tile([128, 128], BF16)
        nc.gpsimd.memset(ident_bf, 0.0)
        nc.gpsimd.affine_select(
            out=ident_bf, in_=ident_bf,
            compare_op=mybir.AluOpType.not_equal, fill=1.0,
            base=0, pattern=[[-1, 128]], channel_multiplier=1,
        )
        sbsa_s = consts.tile([128, 2], F32)
        nc.sync.dma_start(out=sbsa_s, in_=ins["sbsa"])
        if has_bias:
            bias_bc = consts.tile([128, DIM], F32)
            nc.gpsimd.dma_start(
                out=bias_bc, in_=ins["biasout_row"].to_broadcast([128, DIM])
            )
        if has_ab:
            ones_row = consts.tile([1, 128], BF16)
            nc.vector.memset(ones_row, 1.0)
            ab_s = consts.tile([1, DIM], BF16)
            nc.sync.dma_start(out=ab_s, in_=ins["ab_row"])

        # ---- residents ----
        xv_res = resident.tile([128, NCHUNK, XW], F32R)
        e_all = resident.tile([128, NCHUNK, DIM], BF16)
        psbd = resident.tile([128, 2, 128], BF16)
        et_pre = resident.tile([128, 8, 2, 128], BF16)  # E^T, chunks 0..7

        with (
            tc.tile_pool(name="g_psum", bufs=1, space="PSUM") as gp,
            tc.tile_pool(name="fin_sbuf", bufs=1) as fin,
        ):
            g0 = gp.tile([128, XW], F32, tag="g0")
            g1 = gp.tile([128, XW], F32, tag="g1")

            # ================= Pass A =================
            pa_ctx = (
                tc.tile_pool(name="pa_sbuf", bufs=nbufs),
                tc.tile_pool(name="pa_psum", bufs=2, space="PSUM"),
            )
            pa, pap = pa_ctx[0].__enter__(), pa_ctx[1].__enter__()
            pend = []
            for sc in range(NSUPER):
                if sc == 0:
                    zt_s, xt_s = zt_s0, xt_s0
                else:
                    zt_s = pa.tile([128, 2, SUPER * CHUNK], BF16, tag="zt_s")
                    nc.sync.dma_start(out=zt_s, in_=zt_m[sc])
                    xt_s = pa.tile([128, 2, SUPER * CHUNK], BF16, tag="xt_s")
                    nc.sync.dma_start(out=xt_s, in_=xt_m[sc])
                for cp in range(SUPER // 2):
                    c = sc * SUPER + 2 * cp
                    lo = 2 * cp * CHUNK
                    # attn = z @ M (+ab), one PSUM group per chunk
                    attn_ps = pap.tile([128, 2, DIM], F32, tag="attn_ps")
                    for j in range(2):
                        sl = slice(lo + j * CHUNK, lo + (j + 1) * CHUNK)
                        nc.tensor.matmul(
                            attn_ps[:, j, :], zt_s[:, 0, sl], mq_s[:, 0, :],
                            start=True, stop=False,
                        )
                        nc.tensor.matmul(
                            attn_ps[:, j, :], zt_s[:, 1, sl], mq_s[:, 1, :],
                            start=False, stop=not has_ab,
                        )
                        if has_ab:
                            nc.tensor.matmul(
                                attn_ps[:, j, :], ones_row, ab_s,
                                start=False, stop=True,
                            )
                    # E = exp(attn), bf16 resident
                    nc.scalar.activation(e_all[:, c : c + 2, :], attn_ps, Exp)
                    # xv = x @ (sa*Wv)
                    xv_ps = pap.tile([128, 2, DIM], F32, tag="xv_ps")
                    for j in range(2):
                        sl = slice(lo + j * CHUNK, lo + (j + 1) * CHUNK)
                        nc.tensor.matmul(
                            xv_ps[:, j, :], xt_s[:, 0, sl], swv_s[:, 0, :],
                            start=True, stop=False,
                        )
                        nc.tensor.matmul(
                            xv_ps[:, j, :], xt_s[:, 1, sl], swv_s[:, 1, :],
                            start=False, stop=True,
                        )
                    xv_dst = bass.AP(
                        tensor=xv_res.tensor,
                        offset=xv_res.offset + c * XW,
                        ap=[xv_res.ap[0], [XW, 2], [1, DIM]],
                    )
                    if has_bias:
                        bias_bc2 = bass.AP(
                            tensor=bias_bc.tensor,
                            offset=bias_bc.offset,
                            ap=[bias_bc.ap[0], [0, 2], [1, DIM]],
                        )
                        nc.vector.tensor_tensor(
                            out=xv_dst, in0=xv_ps, in1=bias_bc2,
                            op=mybir.AluOpType.add,
                        )
                    elif cp % 2 == 0:
                        nc.vector.tensor_copy(xv_dst, xv_ps)
                    else:
                        nc.scalar.copy(xv_dst, xv_ps)
                # normalization groups: quads normally, pairs on the final
                # super so the end-of-pass drain chain is shorter
                gsz = 2 if sc == NSUPER - 1 else 4
                for q in range(SUPER // gsz):
                    c0 = sc * SUPER + gsz * q
                    # rowsums rs into aux cols 256..259 (per chunk, per head)
                    with nc.allow_low_precision(reason="damped v-path"):
                        nc.vector.tensor_reduce(
                            bass.AP(
                                tensor=xv_res.tensor,
                                offset=xv_res.offset + c0 * XW + DIM,
                                ap=[xv_res.ap[0], [XW, gsz], [1, HEAD]],
                            ),
                            e_all[:, c0 : c0 + gsz, :].rearrange(
                                "p c (h r) -> p c h r", h=HEAD
                            ),
                            axis=mybir.AxisListType.X,
                            op=mybir.AluOpType.add,
                        )
                    rcp = pa.tile([128, 4, HEAD], F32, tag="rcp")
                    nc.vector.reciprocal(
                        rcp[:, 0:gsz, :],
                        bass.AP(
                            tensor=xv_res.tensor,
                            offset=xv_res.offset + c0 * XW + DIM,
                            ap=[xv_res.ap[0], [XW, gsz], [1, HEAD]],
                        ),
                    )
                    eh = pa.tile([128, 4, HEAD, RANK], F32R, tag="eh")
                    rcp_bc = bass.AP(
                        tensor=rcp.tensor,
                        offset=rcp.offset,
                        ap=[rcp.ap[0], [4, gsz], [1, 4], [0, RANK]],
                    )
                    eh_eng = nc.vector
                    eh_eng.tensor_tensor(
                        out=eh[:, 0:gsz],
                        in0=e_all[:, c0 : c0 + gsz, :].rearrange(
                            "p c (h r) -> p c h r", h=HEAD
                        ),
                        in1=rcp_bc,
                        op=mybir.AluOpType.mult,
                    )
                    pend.append((c0, eh, gsz))
                    while len(pend) > 1:
                        cq, ehq, nch = pend.pop(0)
                        for j in range(nch):
                            ehf = ehq[:, j, :, :].rearrange("p h r -> p (h r)")
                            for gi, g in enumerate((g0, g1)):
                                nc.tensor.matmul(
                                    g[:, 0:XW],
                                    ehf[:, gi * 128 : (gi + 1) * 128],
                                    xv_res[:, cq + j, :],
                                    start=(cq + j == 0),
                                    stop=(cq + j == NCHUNK - 1),
                                )
                if sc == 0:
                    # pre-compute E^T for pass B's first two quads while the
                    # PE is still DMA-starved
                    for q4 in range(2):
                        etp_ps = pap.tile([128, 4, 2, 128], BF16, tag="etp_ps")
                        for j in range(4):
                            for kt in range(2):
                                nc.tensor.transpose(
                                    etp_ps[:, j, kt, :],
                                    e_all[:, 4 * q4 + j, kt * 128 : (kt + 1) * 128],
                                    ident_bf,
                                )
                        nc.scalar.copy(et_pre[:, 4 * q4 : 4 * q4 + 4], etp_ps)
            for cq, ehq, nch in pend:
                for j in range(nch):
                    ehf = ehq[:, j, :, :].rearrange("p h r -> p (h r)")
                    for gi, g in enumerate((g0, g1)):
                        nc.tensor.matmul(
                            g[:, 0:XW],
                            ehf[:, gi * 128 : (gi + 1) * 128],
                            xv_res[:, cq + j, :],
                            start=(cq + j == 0),
                            stop=(cq + j == NCHUNK - 1),
                        )
            pend.clear()
            pa_ctx[1].__exit__(None, None, None)
            pa_ctx[0].__exit__(None, None, None)

            # ================= Finalize =================
            # PS rows (h,r): sb_h/sa_h * G[(h,r), :256] / colsum_h, block-diag.
            # Reads G straight from PSUM (DVE only: gpsimd cannot touch PSUM).
            # This is a serial bubble between passes, so keep it minimal.
            nc.gpsimd.memset(psbd, 0.0)
            for gi, g in enumerate((g0, g1)):
                eng = nc.vector
                h0, h1 = 2 * gi, 2 * gi + 1
                cs = fin.tile([128, 1], F32, tag=f"cs{gi}")
                eng.tensor_copy(cs[0:64, :], g[0:64, DIM + h0 : DIM + h0 + 1])
                eng.tensor_copy(cs[64:128, :], g[64:128, DIM + h1 : DIM + h1 + 1])
                rcs = fin.tile([128, 1], F32, tag=f"rcs{gi}")
                nc.vector.reciprocal(rcs, cs)
                eng.tensor_mul(rcs, rcs, sbsa_s[:, gi : gi + 1])
                # block-diag: rows 0:64 (even head) -> first 64-col block of
                # this half; rows 64:128 (odd head) -> second 64-col block
                eng.tensor_scalar_mul(
                    psbd[0:64, gi, 0:64], g[0:64, h0 * 64 : h0 * 64 + 64],
                    rcs[0:64, :],
                )
                eng.tensor_scalar_mul(
                    psbd[64:128, gi, 64:128],
                    g[64:128, h1 * 64 : h1 * 64 + 64],
                    rcs[64:128, :],
                )

        # ================= Pass B =================
        with (
            tc.tile_pool(name="pb_sbuf", bufs=5) as pb,
            tc.tile_pool(name="pb_psum", bufs=2, space="PSUM") as pbp,
        ):
            o_m = out.rearrange("(s j p) f -> s p j f", p=128, j=4)
            for sq in range(NCHUNK // 4):
                c = sq * 4
                if sq < 2:
                    et = et_pre[:, c : c + 4]
                else:
                    # E^T for this quad via PE transpose
                    et_ps = pbp.tile([128, 4, 2, 128], BF16, tag="et_ps")
                    for j in range(4):
                        for kt in range(2):
                            nc.tensor.transpose(
                                et_ps[:, j, kt, :],
                                e_all[:, c + j, kt * 128 : (kt + 1) * 128],
                                ident_bf,
                            )
                    et = pb.tile([128, 4, 2, 128], BF16, tag="et")
                    nc.scalar.copy(et, et_ps)
                out_ps = pbp.tile([128, 4, DIM], F32, tag="out_ps")
                for j in range(4):
                    nc.tensor.matmul(
                        out_ps[:, j, 0:128], et[:, j, 0, :],
                        psbd[:, 0, :], start=True, stop=True,
                    )
                    nc.tensor.matmul(
                        out_ps[:, j, 128:256], et[:, j, 1, :],
                        psbd[:, 1, :], start=True, stop=True,
                    )
                # out = xv + E @ PS, one batched add + one DMA per quad
                ostage = pb.tile([128, 4, DIM], F32, tag="ostage")
                xv_src = bass.AP(
                    tensor=xv_res.tensor,
                    offset=xv_res.offset + c * XW,
                    ap=[xv_res.ap[0], [XW, 4], [1, DIM]],
                )
                nc.vector.tensor_tensor(
                    out=ostage,
                    in0=out_ps,
                    in1=xv_src.bitcast(F32),
                    op=mybir.AluOpType.add,
                )
                nc.sync.dma_start(out=o_m[sq], in_=ostage)


def fold_params(Wq, bq, K, Wv, bv, alpha, beta):
    """Host-side folding of the tiny parameter tensors (all O(256^2))."""
    Wq = np.asarray(Wq, np.float64)
    bq = np.asarray(bq, np.float64)
    K = np.asarray(K, np.float64)
    Wv = np.asarray(Wv, np.float64)
    bv = np.asarray(bv, np.float64)
    sa = 1.0 / (1.0 + np.exp(-np.asarray(alpha, np.float64)[:, 0]))  # (HEAD,)
    sb = 1.0 / (1.0 + np.exp(-np.asarray(beta, np.float64)[:, 0]))
    scale = 1.0 / math.sqrt(HDIM)
    M = np.zeros((DIM, HEAD * RANK))
    ab = np.zeros((HEAD * RANK,))
    for h in range(HEAD):
        Kh = K[:, h, :]
        M[:, h * RANK : (h + 1) * RANK] = (
            Wq[:, h * HDIM : (h + 1) * HDIM] @ Kh.T * scale
        )
        ab[h * RANK : (h + 1) * RANK] = (bq[h * HDIM : (h + 1) * HDIM] @ Kh.T) * scale
    sa_vec = np.repeat(sa, HDIM)  # (256,)
    swv = Wv * sa_vec[None, :]
    biasout = bv * sa_vec
    # sb/sa per PS row: g0 rows = heads (0,1), g1 rows = heads (2,3)
    sbsa = np.zeros((128, 2))
    for gi in range(2):
        sbsa[0:64, gi] = sb[2 * gi] / sa[2 * gi]
        sbsa[64:128, gi] = sb[2 * gi + 1] / sa[2 * gi + 1]
    return {
        "mq": M.astype(np.float32),
        "ab": ab.astype(np.float32),
        "swv": swv.astype(np.float32),
        "biasout_row": biasout.astype(np.float32).reshape(1, DIM),
        "sbsa": sbsa.astype(np.float32),
    }


def build_nc(has_ab, has_bias=True):
    nc = bacc.Bacc("TRN2", target_bir_lowering=False, debug=False,
                   enable_asserts=False)
    ins = {
        "zt": nc.dram_tensor("zt", [DIM, N], BF16, kind="ExternalInput").ap(),
        "xt": nc.dram_tensor("xt", [DIM, N], BF16, kind="ExternalInput").ap(),
        "mq": nc.dram_tensor("mq", [DIM, DIM], BF16, kind="ExternalInput").ap(),
        "swv": nc.dram_tensor("swv", [DIM, DIM], BF16, kind="ExternalInput").ap(),
        "sbsa": nc.dram_tensor("sbsa", [128, 2], F32, kind="ExternalInput").ap(),
        "biasout_row": (
            nc.dram_tensor("biasout_row", [1, DIM], F32, kind="ExternalInput").ap()
            if has_bias
            else None
        ),
        "ab_row": (
            nc.dram_tensor("ab_row", [1, DIM], BF16, kind="ExternalInput").ap()
            if has_ab
            else None
        ),
    }
    ins["has_bias"] = has_bias
    outs = {"out": nc.dram_tensor("out", [N, DIM], F32, kind="ExternalOutput").ap()}
    reps = int(os.environ.get("KREPS", "1"))
    with tile.TileContext(nc) as tc:
        for _ in range(reps):
            build_body(tc, outs, ins)
    nc.compile()
    return nc


LAST_RESULTS = None


def prepare(x, z, Wq, bq, K, Wv, bv, alpha, beta):
    """Build the compiled module and per-core input maps."""
    import ml_dtypes

    x = np.asarray(x, np.float32)
    z = np.asarray(z, np.float32)
    p = fold_params(Wq, bq, K, Wv, bv, alpha, beta)
    has_ab = bool(np.any(p["ab"] != 0.0))
    has_bias = bool(np.any(p["biasout_row"] != 0.0))

    nc = build_nc(has_ab, has_bias)

    common = {
        "mq": p["mq"].astype(ml_dtypes.bfloat16),
        "swv": p["swv"].astype(ml_dtypes.bfloat16),
        "sbsa": p["sbsa"],
    }
    if has_bias:
        common["biasout_row"] = p["biasout_row"]
    if has_ab:
        common["ab_row"] = p["ab"].reshape(1, DIM).astype(ml_dtypes.bfloat16)

    in_maps = [
        dict(
            common,
            zt=z[i].T.astype(ml_dtypes.bfloat16),
            xt=x[i].T.astype(ml_dtypes.bfloat16),
        )
        for i in range(NCORES)
    ]
    return nc, in_maps


def kernel(x, z, Wq, bq, K, Wv, bv, alpha, beta):
    global LAST_RESULTS
    from concourse.bass_utils import run_bass_kernel_spmd

    nc, in_maps = prepare(x, z, Wq, bq, K, Wv, bv, alpha, beta)
    res = run_bass_kernel_spmd(nc, in_maps, core_ids=list(range(NCORES)))
    LAST_RESULTS = res
    out = np.stack([res.results[i]["out"] for i in range(NCORES)], axis=0)
    return out.astype(np.float32)


# revision 9
# speedup vs baseline: 1.4111x; 1.4111x over previous
# Trainium2 Bass kernel v2 for nn_Lowrank_Spattention (sparse_attention).
#
# Key change vs v1: z and x are transposed AND cast to bf16 ON HOST,
# so attn = z@M and xv = x@Wv run directly with DMA'd f32r tiles as lhsT —
# no PE transposes of z/x, no PSUM->SBUF copies for them, no bf16 casts.
#
# Per core (batch element), n=8192 tokens, f=256, h=4 heads, r=64 ranks:
#   attn = z @ M            (M = Wq_h K_h^T/sqrt(d) folded, f32r)
#   E    = exp(attn)        (bf16 resident, row layout)
#   xv   = sa * (x @ Wv)    (f32r resident, + sa*bv if bias)
#   G    = Eh^T @ [xv | rs] (PSUM accum over all chunks; Eh = E/rowsum)
#   PS   = (sb/sa) * G_blockdiag / colsum   (tiny finalize)
#   out  = xv + E @ PS      (pass B; eT via PE transpose)
#
# aux cols 256..259 of xv_res hold per-head rowsums rs so that
# G[:, 256+h] = sum_n Eh[n,(h,r)]*rs[n,h] = colsum_E for rows of head h.

import math
import os

import numpy as np

import concourse.bass as bass
import concourse.mybir as mybir
import concourse.tile as tile
from concourse import bacc

B, N, DIM = 8, 8192, 256
HEAD, RANK, HDIM = 4, 64, 64
NCORES = 8
CHUNK = 128
NCHUNK = N // CHUNK          # 64
SUPER = 8                    # chunks per staging super-chunk
NSUPER = NCHUNK // SUPER     # 8
XW = DIM + HEAD              # 260: [xv | rs0..rs3]

F32 = mybir.dt.float32
F32R = mybir.dt.float32r
BF16 = mybir.dt.bfloat16
Exp = mybir.ActivationFunctionType.Exp


def build_body(tc, outs, ins):
    nc = tc.nc
    zt, xt = ins["zt"], ins["xt"]
    out = outs["out"]
    has_ab = ins.get("ab_row") is not None
    has_bias = bool(ins.get("has_bias", False))
    nbufs = 3

    with (
        tc.tile_pool(name="consts", bufs=1) as consts,
        tc.tile_pool(name="resident", bufs=1) as resident,
        tc.tile_pool(name="prefetch", bufs=1) as prefetch,
    ):
        # ---- prefetch the first chunks of zt/xt before anything else, so
        # the PE pipeline starts as early as possible ----
        zt_m = zt.rearrange("(t p) (s c) -> s p t c", p=128, c=SUPER * CHUNK)
        xt_m = xt.rearrange("(t p) (s c) -> s p t c", p=128, c=SUPER * CHUNK)
        zt_s0 = prefetch.tile([128, 2, SUPER * CHUNK], BF16)
        nc.sync.dma_start(out=zt_s0[:, :, 0:CHUNK], in_=zt_m[0, :, :, 0:CHUNK])
        mq_s = consts.tile([128, 2, DIM], BF16)
        nc.sync.dma_start(out=mq_s, in_=ins["mq"].rearrange("(t p) n -> p t n", p=128))
        xt_s0 = prefetch.tile([128, 2, SUPER * CHUNK], BF16)
        nc.sync.dma_start(out=xt_s0[:, :, 0:CHUNK], in_=xt_m[0, :, :, 0:CHUNK])
        swv_s = consts.tile([128, 2, DIM], BF16)
        nc.sync.dma_start(out=swv_s, in_=ins["swv"].rearrange("(t p) n -> p t n", p=128))
        c4 = 4 * CHUNK
        nc.sync.dma_start(out=zt_s0[:, :, CHUNK:c4], in_=zt_m[0, :, :, CHUNK:c4])
        nc.sync.dma_start(out=xt_s0[:, :, CHUNK:c4], in_=xt_m[0, :, :, CHUNK:c4])
        nc.sync.dma_start(out=zt_s0[:, :, c4:], in_=zt_m[0, :, :, c4:])
        nc.sync.dma_start(out=xt_s0[:, :, c4:], in_=xt_m[0, :, :, c4:])

        # ---- constants ----
        ident_bf = consts.tile([128, 128], BF16)
        nc.gpsimd.memset(ident_bf, 0.0)
        nc.gpsimd.affine_select(
            out=ident_bf, in_=ident_bf,
            compare_op=mybir.AluOpType.not_equal, fill=1.0,
            base=0, pattern=[[-1, 128]], channel_multiplier=1,
        )
        sbsa_s = consts.tile([128, 2], F32)
        nc.sync.dma_start(out=sbsa_s, in_=ins["sbsa"])
        if has_bias:
            bias_bc = consts.tile([128, DIM], F32)
            nc.gpsimd.dma_start(
                out=bias_bc, in_=ins["biasout_row"].to_broadcast([128, DIM])
            )
        if has_ab:
            ones_row = consts.tile([1, 128], BF16)
            nc.vector.memset(ones_row, 1.0)
            ab_s = consts.tile([1, DIM], BF16)
            nc.sync.dma_start(out=ab_s, in_=ins["ab_row"])

        # ---- residents ----
        xv_res = resident.tile([128, NCHUNK, XW], F32R)
        e_all = resident.tile([128, NCHUNK, DIM], BF16)
        psbd = resident.tile([128, 2, 128], BF16)
        et_pre = resident.tile([128, 8, 2, 128], BF16)  # E^T, chunks 0..7

        with (
            tc.tile_pool(name="g_psum", bufs=1, space="PSUM") as gp,
            tc.tile_pool(name="fin_sbuf", bufs=1) as fin,
        ):
            g0 = gp.tile([128, XW], F32, tag="g0")
            g1 = gp.tile([128, XW], F32, tag="g1")

            # ================= Pass A =================
            pa_ctx = (
                tc.tile_pool(name="pa_sbuf", bufs=nbufs),
                tc.tile_pool(name="pa_psum", bufs=2, space="PSUM"),
            )
            pa, pap = pa_ctx[0].__enter__(), pa_ctx[1].__enter__()
            pend = []
            for sc in range(NSUPER):
                if sc == 0:
                    zt_s, xt_s = zt_s0, xt_s0
                else:
                    zt_s = pa.tile([128, 2, SUPER * CHUNK], BF16, tag="zt_s")
                    nc.sync.dma_start(out=zt_s, in_=zt_m[sc])
                    xt_s = pa.tile([128, 2, SUPER * CHUNK], BF16, tag="xt_s")
                    nc.sync.dma_start(out=xt_s, in_=xt_m[sc])
                for cp in range(SUPER // 2):
                    c = sc * SUPER + 2 * cp
                    lo = 2 * cp * CHUNK
                    # attn = z @ M (+ab), one PSUM group per chunk
                    attn_ps = pap.tile([128, 2, DIM], F32, tag="attn_ps")
                    for j in range(2):
                        sl = slice(lo + j * CHUNK, lo + (j + 1) * CHUNK)
                        nc.tensor.matmul(
                            attn_ps[:, j, :], zt_s[:, 0, sl], mq_s[:, 0, :],
                            start=True, stop=False,
                        )
                        nc.tensor.matmul(
                            attn_ps[:, j, :], zt_s[:, 1, sl], mq_s[:, 1, :],
                            start=False, stop=not has_ab,
                        )
                        if has_ab:
                            nc.tensor.matmul(
                                attn_ps[:, j, :], ones_row, ab_s,
                                start=False, stop=True,
                            )
                    # E = exp(attn), bf16 resident
                    nc.scalar.activation(e_all[:, c : c + 2, :], attn_ps, Exp)
                    # xv = x @ (sa*Wv)
                    xv_ps = pap.tile([128, 2, DIM], F32, tag="xv_ps")
                    for j in range(2):
                        sl = slice(lo + j * CHUNK, lo + (j + 1) * CHUNK)
                        nc.tensor.matmul(
                            xv_ps[:, j, :], xt_s[:, 0, sl], swv_s[:, 0, :],
                            start=True, stop=False,
                        )
                        nc.tensor.matmul(
                            xv_ps[:, j, :], xt_s[:, 1, sl], swv_s[:, 1, :],
                            start=False, stop=True,
                        )
                    xv_dst = bass.AP(
                        tensor=xv_res.tensor,
                        offset=xv_res.offset + c * XW,
                        ap=[xv_res.ap[0], [XW, 2], [1, DIM]],
                    )
                    if has_bias:
                        bias_bc2 = bass.AP(
                            tensor=bias_bc.tensor,
                            offset=bias_bc.offset,
                            ap=[bias_bc.ap[0], [0, 2], [1, DIM]],
                        )
                        nc.vector.tensor_tensor(
                            out=xv_dst, in0=xv_ps, in1=bias_bc2,
                            op=mybir.AluOpType.add,
                        )
                    else:
                        nc.scalar.copy(xv_dst, xv_ps)
                # normalization groups: quads normally, pairs on the final
                # super so the end-of-pass drain chain is shorter
                gsz = 2 if sc == NSUPER - 1 else 4
                for q in range(SUPER // gsz):
                    c0 = sc * SUPER + gsz * q
                    # rowsums rs into aux cols 256..259 (per chunk, per head)
                    with nc.allow_low_precision(reason="damped v-path"):
                        nc.vector.tensor_reduce(
                            bass.AP(
                                tensor=xv_res.tensor,
                                offset=xv_res.offset + c0 * XW + DIM,
                                ap=[xv_res.ap[0], [XW, gsz], [1, HEAD]],
                            ),
                            e_all[:, c0 : c0 + gsz, :].rearrange(
                                "p c (h r) -> p c h r", h=HEAD
                            ),
                            axis=mybir.AxisListType.X,
                            op=mybir.AluOpType.add,
                        )
                    rcp = pa.tile([128, 4, HEAD], F32, tag="rcp")
                    nc.vector.reciprocal(
                        rcp[:, 0:gsz, :],
                        bass.AP(
                            tensor=xv_res.tensor,
                            offset=xv_res.offset + c0 * XW + DIM,
                            ap=[xv_res.ap[0], [XW, gsz], [1, HEAD]],
                        ),
                    )
                    eh = pa.tile([128, 4, HEAD, RANK], F32R, tag="eh")
                    rcp_bc = bass.AP(
                        tensor=rcp.tensor,
                        offset=rcp.offset,
                        ap=[rcp.ap[0], [4, gsz], [1, 4], [0, RANK]],
                    )
                    eh_eng = nc.vector
                    eh_eng.tensor_tensor(
                        out=eh[:, 0:gsz],
                        in0=e_all[:, c0 : c0 + gsz, :].rearrange(
                            "p c (h r) -> p c h r", h=HEAD
                        ),
                        in1=rcp_bc,
                        op=mybir.AluOpType.mult,
                    )
                    pend.append((c0, eh, gsz))
                    while len(pend) > 1:
                        cq, ehq, nch = pend.pop(0)
                        for j in range(nch):
                            ehf = ehq[:, j, :, :].rearrange("p h r -> p (h r)")
                            for gi, g in enumerate((g0, g1)):
                                nc.tensor.matmul(
                                    g[:, 0:XW],
                                    ehf[:, gi * 128 : (gi + 1) * 128],
                                    xv_res[:, cq + j, :],
                                    start=(cq + j == 0),
                                    stop=(cq + j == NCHUNK - 1),
                                )
                if sc == 0:
                    # pre-compute E^T for pass B's first two quads while the
                    # PE is still DMA-starved
                    for q4 in range(2):
                        etp_ps = pap.tile([128, 4, 2, 128], BF16, tag="etp_ps")
                        for j in range(4):
                            for kt in range(2):
                                nc.tensor.transpose(
                                    etp_ps[:, j, kt, :],
                                    e_all[:, 4 * q4 + j, kt * 128 : (kt + 1) * 128],
                                    ident_bf,
                                )
                        nc.scalar.copy(et_pre[:, 4 * q4 : 4 * q4 + 4], etp_ps)
            for cq, ehq, nch in pend:
                for j in range(nch):
                    ehf = ehq[:, j, :, :].rearrange("p h r -> p (h r)")
                    for gi, g in enumerate((g0, g1)):
                        nc.tensor.matmul(
                            g[:, 0:XW],
                            ehf[:, gi * 128 : (gi + 1) * 128],
                            xv_res[:, cq + j, :],
                            start=(cq + j == 0),
                            stop=(cq + j == NCHUNK - 1),
                        )
            pend.clear()
            pa_ctx[1].__exit__(None, None, None)
            pa_ctx[0].__exit__(None, None, None)

            # ================= Finalize =================
            # PS rows (h,r): sb_h/sa_h * G[(h,r), :256] / colsum_h, block-diag.
            # Reads G straight from PSUM (DVE only: gpsimd cannot touch PSUM).
            # This is a serial bubble between passes, so keep it minimal.
            nc.gpsimd.memset(psbd, 0.0)
            for gi, g in enumerate((g0, g1)):
                eng = nc.vector
                h0, h1 = 2 * gi, 2 * gi + 1
                cs = fin.tile([128, 1], F32, tag=f"cs{gi}")
                eng.tensor_copy(cs[0:64, :], g[0:64, DIM + h0 : DIM + h0 + 1])
                eng.tensor_copy(cs[64:128, :], g[64:128, DIM + h1 : DIM + h1 + 1])
                rcs = fin.tile([128, 1], F32, tag=f"rcs{gi}")
                nc.vector.reciprocal(rcs, cs)
                eng.tensor_mul(rcs, rcs, sbsa_s[:, gi : gi + 1])
                # block-diag: rows 0:64 (even head) -> first 64-col block of
                # this half; rows 64:128 (odd head) -> second 64-col block
                eng.tensor_scalar_mul(
                    psbd[0:64, gi, 0:64], g[0:64, h0 * 64 : h0 * 64 + 64],
                    rcs[0:64, :],
                )
                eng.tensor_scalar_mul(
                    psbd[64:128, gi, 64:128],
                    g[64:128, h1 * 64 : h1 * 64 + 64],
                    rcs[64:128, :],
                )

        # ================= Pass B =================
        with (
            tc.tile_pool(name="pb_sbuf", bufs=5) as pb,
            tc.tile_pool(name="pb_psum", bufs=2, space="PSUM") as pbp,
        ):
            o_m = out.rearrange("(s j p) f -> s p j f", p=128, j=4)
            for sq in range(NCHUNK // 4):
                c = sq * 4
                if sq < 2:
                    et = et_pre[:, c : c + 4]
                else:
                    # E^T for this quad via PE transpose
                    et_ps = pbp.tile([128, 4, 2, 128], BF16, tag="et_ps")
                    for j in range(4):
                        for kt in range(2):
                            nc.tensor.transpose(
                                et_ps[:, j, kt, :],
                                e_all[:, c + j, kt * 128 : (kt + 1) * 128],
                                ident_bf,
                            )
                    et = pb.tile([128, 4, 2, 128], BF16, tag="et")
                    nc.scalar.copy(et, et_ps)
                out_ps = pbp.tile([128, 4, DIM], F32, tag="out_ps")
                for j in range(4):
                    nc.tensor.matmul(
                        out_ps[:, j, 0:128], et[:, j, 0, :],
                        psbd[:, 0, :], start=True, stop=True,
                    )
                    nc.tensor.matmul(
                        out_ps[:, j, 128:256], et[:, j, 1, :],
                        psbd[:, 1, :], start=True, stop=True,
                    )
                # out = xv + E @ PS, one batched add + one DMA per quad
                ostage = pb.tile([128, 4, DIM], BF16, tag="ostage")
                xv_src = bass.AP(
                    tensor=xv_res.tensor,
                    offset=xv_res.offset + c * XW,
                    ap=[xv_res.ap[0], [XW, 4], [1, DIM]],
                )
                nc.vector.tensor_tensor(
                    out=ostage,
                    in0=out_ps,
                    in1=xv_src.bitcast(F32),
                    op=mybir.AluOpType.add,
                )
                nc.sync.dma_start(out=o_m[sq], in_=ostage)


def fold_params(Wq, bq, K, Wv, bv, alpha, beta):
    """Host-side folding of the tiny parameter tensors (all O(256^2))."""
    Wq = np.asarray(Wq, np.float64)
    bq = np.asarray(bq, np.float64)
    K = np.asarray(K, np.float64)
    Wv = np.asarray(Wv, np.float64)
    bv = np.asarray(bv, np.float64)
    sa = 1.0 / (1.0 + np.exp(-np.asarray(alpha, np.float64)[:, 0]))  # (HEAD,)
    sb = 1.0 / (1.0 + np.exp(-np.asarray(beta, np.float64)[:, 0]))
    scale = 1.0 / math.sqrt(HDIM)
    M = np.zeros((DIM, HEAD * RANK))
    ab = np.zeros((HEAD * RANK,))
    for h in range(HEAD):
        Kh = K[:, h, :]
        M[:, h * RANK : (h + 1) * RANK] = (
            Wq[:, h * HDIM : (h + 1) * HDIM] @ Kh.T * scale
        )
        ab[h * RANK : (h + 1) * RANK] = (bq[h * HDIM : (h + 1) * HDIM] @ Kh.T) * scale
    sa_vec = np.repeat(sa, HDIM)  # (256,)
    swv = Wv * sa_vec[None, :]
    biasout = bv * sa_vec
    # sb/sa per PS row: g0 rows = heads (0,1), g1 rows = heads (2,3)
    sbsa = np.zeros((128, 2))
    for gi in range(2):
        sbsa[0:64, gi] = sb[2 * gi] / sa[2 * gi]
        sbsa[64:128, gi] = sb[2 * gi + 1] / sa[2 * gi + 1]
    return {
        "mq": M.astype(np.float32),
        "ab": ab.astype(np.float32),
        "swv": swv.astype(np.float32),
        "biasout_row": biasout.astype(np.float32).reshape(1, DIM),
        "sbsa": sbsa.astype(np.float32),
    }


def build_nc(has_ab, has_bias=True):
    nc = bacc.Bacc("TRN2", target_bir_lowering=False, debug=False,
                   enable_asserts=False)
    ins = {
        "zt": nc.dram_tensor("zt", [DIM, N], BF16, kind="ExternalInput").ap(),
        "xt": nc.dram_tensor("xt", [DIM, N], BF16, kind="ExternalInput").ap(),
        "mq": nc.dram_tensor("mq", [DIM, DIM], BF16, kind="ExternalInput").ap(),
        "swv": nc.dram_tensor("swv", [DIM, DIM], BF16, kind="ExternalInput").ap(),
        "sbsa": nc.dram_tensor("sbsa", [128, 2], F32, kind="ExternalInput").ap(),
        "biasout_row": (
            nc.dram_tensor("biasout_row", [1, DIM], F32, kind="ExternalInput").ap()
            if has_bias
            else None
        ),
        "ab_row": (
            nc.dram_tensor("ab_row", [1, DIM], BF16, kind="ExternalInput").ap()
            if has_ab
            else None
        ),
    }
    ins["has_bias"] = has_bias
    outs = {"out": nc.dram_tensor("out", [N, DIM], BF16, kind="ExternalOutput").ap()}
    reps = int(os.environ.get("KREPS", "1"))
    with tile.TileContext(nc) as tc:
        for _ in range(reps):
            build_body(tc, outs, ins)
    nc.compile()
    return nc


LAST_RESULTS = None


def prepare(x, z, Wq, bq, K, Wv, bv, alpha, beta):
    """Build the compiled module and per-core input maps."""
    import ml_dtypes

    x = np.asarray(x, np.float32)
    z = np.asarray(z, np.float32)
    p = fold_params(Wq, bq, K, Wv, bv, alpha, beta)
    has_ab = bool(np.any(p["ab"] != 0.0))
    has_bias = bool(np.any(p["biasout_row"] != 0.0))

    nc = build_nc(has_ab, has_bias)

    common = {
        "mq": p["mq"].astype(ml_dtypes.bfloat16),
        "swv": p["swv"].astype(ml_dtypes.bfloat16),
        "sbsa": p["sbsa"],
    }
    if has_bias:
        common["biasout_row"] = p["biasout_row"]
    if has_ab:
        common["ab_row"] = p["ab"].reshape(1, DIM).astype(ml_dtypes.bfloat16)

    in_maps = [
        dict(
            common,
            zt=z[i].T.astype(ml_dtypes.bfloat16),
            xt=x[i].T.astype(ml_dtypes.bfloat16),
        )
        for i in range(NCORES)
    ]
    return nc, in_maps


def kernel(x, z, Wq, bq, K, Wv, bv, alpha, beta):
    global LAST_RESULTS
    from concourse.bass_utils import run_bass_kernel_spmd

    nc, in_maps = prepare(x, z, Wq, bq, K, Wv, bv, alpha, beta)
    res = run_bass_kernel_spmd(nc, in_maps, core_ids=list(range(NCORES)))
    LAST_RESULTS = res
    out = np.stack([res.results[i]["out"] for i in range(NCORES)], axis=0)
    return out.astype(np.float32)
